# revision 16
# baseline (speedup 1.0000x reference)
"""Trainium2 Bass kernel for DetectionLayer (refine + per-class NMS).

Contract: kernel(rois, probs, deltas) with FULL inputs
  rois   [16, 4096, 4]   f32
  probs  [16, 4096, 81]  f32
  deltas [16, 4096, 81, 4] f32
returns [16, 100, 6] f32 detections, matching the jax reference.

Sharding: pure data parallel - 2 images per core across 8 NeuronCores.

Structure (optimized for the data-dependent early exit):
  fast path (always): chunked probs load on both HW-DGE queues, per-chunk
    class-max pipelined with the DMA (img0 on Vector, img1 on GpSimd),
    one fused confidence-gate count, one values_load, early zero-fill of
    the output.
  tc.If(count > 0): deltas+rois load, exact per-argmax delta select, box
    refine, then the merged NMS For_i (both images per iteration) and the
    real output DMAs. When no roi passes min-confidence the whole branch
    is skipped by a register compare - no loop-exit ladders, no DMA.
"""

import numpy as np

import concourse.bacc as bacc
import concourse.bass as bass
import concourse.bass_isa as bass_isa
import concourse.mybir as mybir
from concourse.expressions import smin
from concourse.tile import TileContext

B = 16              # full batch
NCORES = 8
BPC = B // NCORES   # images per core
N = 4096            # rois per image
C = 81              # classes
K = 100             # detection_max_instances
P = 128             # SBUF partitions
NP = N // P         # rois per partition (32)
CHUNKS = (8, 8, 8, 4, 4)  # probs DMA chunk sizes (rois per partition); the
                          # tail chunks shrink so the last class-max is off
                          # the DMA critical path
NEG = -1e9
MIN_CONF = 0.7
NMS_T = 0.3
F32 = mybir.dt.float32
I32 = mybir.dt.int32
U8 = mybir.dt.uint8


def _refine_image(nc, tc, pools, img, rois_t, deltas_t, state):
    """Deltas select, box refine, NMS-state build for one image.
    Emitted inside the If(count > 0) branch."""
    cpool, big, sm = pools
    pt = state[f"probs{img}"]
    scores = state["scores"][:, img]   # [P, NP]
    ge = state["ge"][:, img]           # [P, NP]
    crev = state["crev"]
    negs = state["negs"]
    sc = state[f"sc{img}"]
    ob = state[f"ob{img}"]
    ar = state[f"ar{img}"]
    cat = state[f"cat{img}"]

    rois_ap = rois_t[img].rearrange("(p n) k -> p n k", p=P)          # [128,32,4]
    deltas_ap = deltas_t[img].rearrange("(p n) c k -> p n c k", p=P)

    dt_ = big.tile([P, NP, C, 4], F32, tag=f"deltas{img}", name=f"deltas{img}")
    # split the 5.3MB load across both HW-DGE queues
    for s in range(8):
        sl = slice(16 * s, 16 * s + 16)
        eng = nc.sync if (s % 2 == 0) else nc.scalar
        eng.dma_start(out=dt_[sl], in_=deltas_ap[sl])
    rt = sm.tile([P, NP, 4], F32, tag=f"rois{img}", name=f"rois{img}")
    nc.sync.dma_start(out=rt, in_=rois_ap)

    # class id = 80 - max((80-c) * (probs == score))  (ties -> smallest c,
    # like argmax). m is built in place over the probs tile.
    m = pt
    nc.vector.tensor_tensor(
        m, pt, scores.unsqueeze(2).to_broadcast([P, NP, C]),
        op=mybir.AluOpType.is_equal,
    )
    nc.vector.tensor_tensor(m, m, crev, op=mybir.AluOpType.mult)
    cid = sm.tile([P, NP], F32, tag=f"cid{img}", name=f"cid{img}")
    nc.vector.reduce_max(cid, m, axis=mybir.AxisListType.X)
    nc.vector.tensor_scalar(
        out=cid, in0=cid, scalar1=-1.0, scalar2=float(C - 1),
        op0=mybir.AluOpType.mult, op1=mybir.AluOpType.add,
    )

    # exact one-hot at the argmax class (safe under intra-roi ties):
    # m2 = (crev == 80 - cid), rebuilt over the probs tile
    t80 = sm.tile([P, NP], F32, tag=f"t80{img}", name=f"t80{img}")
    nc.vector.tensor_scalar(
        out=t80, in0=cid, scalar1=-1.0, scalar2=float(C - 1),
        op0=mybir.AluOpType.mult, op1=mybir.AluOpType.add,
    )
    nc.vector.tensor_tensor(
        m, crev, t80.unsqueeze(2).to_broadcast([P, NP, C]),
        op=mybir.AluOpType.is_equal,
    )

    # select argmax-class delta: deltas *= m2 (bcast over k), sum over c
    d_perm = dt_.rearrange("p n c k -> p n k c")
    nc.vector.tensor_tensor(
        d_perm, d_perm, m.unsqueeze(2).to_broadcast([P, NP, 4, C]),
        op=mybir.AluOpType.mult,
    )
    dsel = sm.tile([P, NP, 4], F32, tag=f"dsel{img}", name=f"dsel{img}")
    nc.vector.reduce_sum(dsel, d_perm, axis=mybir.AxisListType.X)

    # bbox_std scaling (match reference op order exactly)
    nc.vector.tensor_scalar_mul(dsel[:, :, 0:2], dsel[:, :, 0:2], 0.1)
    nc.vector.tensor_scalar_mul(dsel[:, :, 2:4], dsel[:, :, 2:4], 0.2)

    # ---- apply deltas + clip (mirrors _apply_deltas fp32 op order) ----
    h = sm.tile([P, NP], F32, tag=f"h{img}", name=f"h{img}")
    w = sm.tile([P, NP], F32, tag=f"w{img}", name=f"w{img}")
    nc.vector.tensor_sub(h, rt[:, :, 2], rt[:, :, 0])
    nc.vector.tensor_sub(w, rt[:, :, 3], rt[:, :, 1])
    t1 = sm.tile([P, NP], F32, tag=f"t1{img}", name=f"t1{img}")
    t2 = sm.tile([P, NP], F32, tag=f"t2{img}", name=f"t2{img}")
    cy = sm.tile([P, NP], F32, tag=f"cy{img}", name=f"cy{img}")
    cx = sm.tile([P, NP], F32, tag=f"cx{img}", name=f"cx{img}")
    # cy = y1 + 0.5*h + dy*h
    nc.vector.tensor_scalar_mul(t1, h, 0.5)
    nc.vector.tensor_add(t2, rt[:, :, 0], t1)
    nc.vector.tensor_mul(t1, dsel[:, :, 0], h)
    nc.vector.tensor_add(cy, t2, t1)
    # cx = x1 + 0.5*w + dx*w
    nc.vector.tensor_scalar_mul(t1, w, 0.5)
    nc.vector.tensor_add(t2, rt[:, :, 1], t1)
    nc.vector.tensor_mul(t1, dsel[:, :, 1], w)
    nc.vector.tensor_add(cx, t2, t1)
    # h *= exp(dh); w *= exp(dw)
    e = sm.tile([P, NP], F32, tag=f"e{img}", name=f"e{img}")
    nc.scalar.activation(e, dsel[:, :, 2], mybir.ActivationFunctionType.Exp)
    nc.vector.tensor_mul(h, h, e)
    nc.scalar.activation(e, dsel[:, :, 3], mybir.ActivationFunctionType.Exp)
    nc.vector.tensor_mul(w, w, e)

    ref = sm.tile([P, NP, 4], F32, tag=f"ref{img}", name=f"ref{img}")
    nc.vector.tensor_scalar_mul(t1, h, 0.5)
    nc.vector.tensor_sub(ref[:, :, 0], cy, t1)
    nc.vector.tensor_add(ref[:, :, 2], cy, t1)
    nc.vector.tensor_scalar_mul(t2, w, 0.5)
    nc.vector.tensor_sub(ref[:, :, 1], cx, t2)
    nc.vector.tensor_add(ref[:, :, 3], cx, t2)
    nc.vector.tensor_scalar(
        out=ref, in0=ref, scalar1=0.0, scalar2=1.0,
        op0=mybir.AluOpType.max, op1=mybir.AluOpType.min,
    )

    # ---- NMS state ----
    # valid = (cid > 0) & (score >= MIN_CONF); sc0 = valid ? score : NEG
    vf = sm.tile([P, NP], F32, tag=f"vf{img}", name=f"vf{img}")
    nc.vector.tensor_single_scalar(vf, cid, 0.5, op=mybir.AluOpType.is_ge)
    v = sm.tile([P, NP], U8, tag=f"v{img}", name=f"v{img}")
    nc.vector.tensor_mul(v, vf, ge)
    nc.vector.tensor_copy(sc, negs)
    nc.vector.copy_predicated(sc, v, scores)

    # offset boxes = ref + 2*cid, per-class NMS trick
    nc.vector.scalar_tensor_tensor(
        out=ob, in0=cid.unsqueeze(2).to_broadcast([P, NP, 4]), scalar=2.0,
        in1=ref, op0=mybir.AluOpType.mult, op1=mybir.AluOpType.add,
    )
    # areas of offset boxes
    ar2 = sm.tile([P, NP, 2], F32, tag=f"ar2{img}", name=f"ar2{img}")
    nc.vector.tensor_sub(ar2, ob[:, :, 2:4], ob[:, :, 0:2])
    nc.vector.tensor_mul(ar, ar2[:, :, 0], ar2[:, :, 1])
    # cat = [ref(4), cid, score] for one-shot row extraction
    nc.vector.tensor_copy(cat[:, :, 0:4], ref)
    nc.vector.tensor_copy(cat[:, :, 4], cid)
    nc.vector.tensor_copy(cat[:, :, 5], scores)


def _nms_iter(nc, tc, pools, img, state, i, det, det_off):
    """One greedy-NMS step for one image (body of the merged NMS loop).
    Tie-break matches argmax: among equal-score candidates the lowest
    global roi index wins (selected via max over negated indices)."""
    cpool, big, sm = pools
    negs = state["negs"]
    gidxn = state["gidxn"]
    sc = state[f"sc{img}"]
    ob = state[f"ob{img}"]
    ar = state[f"ar{img}"]
    cat = state[f"cat{img}"]

    pm = sm.tile([P, 1], F32, tag=f"pm{img}", name=f"pm{img}")
    nc.vector.reduce_max(pm, sc, axis=mybir.AxisListType.X)
    gm = sm.tile([P, 1], F32, tag=f"gm{img}", name=f"gm{img}")
    nc.gpsimd.partition_all_reduce(gm, pm, channels=P,
                                   reduce_op=bass_isa.ReduceOp.max)
    # candidates at the max score (may be several on exact ties)
    msku = sm.tile([P, NP], U8, tag=f"msku{img}", name=f"msku{img}")
    nc.vector.tensor_tensor(msku, sc, gm.to_broadcast([P, NP]),
                            op=mybir.AluOpType.is_equal)
    # pick lowest roi index among candidates: max over (-index | candidate)
    cand = sm.tile([P, NP], F32, tag=f"cand{img}", name=f"cand{img}")
    nc.vector.tensor_copy(cand, negs)
    nc.vector.copy_predicated(cand, msku, gidxn)
    cmx = sm.tile([P, 1], F32, tag=f"cmx{img}", name=f"cmx{img}")
    nc.vector.reduce_max(cmx, cand, axis=mybir.AxisListType.X)
    gmx = sm.tile([P, 1], F32, tag=f"gmx{img}", name=f"gmx{img}")
    nc.gpsimd.partition_all_reduce(gmx, cmx, channels=P,
                                   reduce_op=bass_isa.ReduceOp.max)
    # exact one-hot of the selected candidate
    msk2 = sm.tile([P, NP], F32, tag=f"msk2{img}", name=f"msk2{img}")
    nc.vector.tensor_tensor(msk2, gidxn, gmx.to_broadcast([P, NP]),
                            op=mybir.AluOpType.is_equal)
    # extract its [ref, cid, score] row via masked sum
    mb6 = sm.tile([P, NP, 6], F32, tag=f"mb6{img}", name=f"mb6{img}")
    nc.vector.tensor_tensor(
        mb6, cat, msk2.unsqueeze(2).to_broadcast([P, NP, 6]),
        op=mybir.AluOpType.mult,
    )
    r6p = sm.tile([P, 6], F32, tag=f"r6p{img}", name=f"r6p{img}")
    nc.vector.reduce_sum(r6p, mb6.rearrange("p n k -> p k n"),
                         axis=mybir.AxisListType.X)
    r6 = sm.tile([P, 6], F32, tag=f"r6{img}", name=f"r6{img}")
    nc.gpsimd.partition_all_reduce(r6, r6p, channels=P,
                                   reduce_op=bass_isa.ReduceOp.add)
    # zero the row when scores are exhausted (gm == NEG)
    okm = sm.tile([P, 1], F32, tag=f"okm{img}", name=f"okm{img}")
    nc.vector.tensor_single_scalar(okm, gm, NEG * 0.5,
                                   op=mybir.AluOpType.is_gt)
    nc.vector.tensor_mul(r6, r6, okm.to_broadcast([P, 6]))
    nc.vector.tensor_copy(det[0:1, bass.ds(det_off + i * 6, 6)], r6[0:1, :])

    # selected offset box, replicated on all partitions
    sb = sm.tile([P, 4], F32, tag=f"sb{img}", name=f"sb{img}")
    nc.vector.scalar_tensor_tensor(
        out=sb, in0=r6[:, 4:5].to_broadcast([P, 4]), scalar=2.0,
        in1=r6[:, 0:4], op0=mybir.AluOpType.mult, op1=mybir.AluOpType.add,
    )
    # IoU(selected, all) on offset boxes
    mx = sm.tile([P, NP, 2], F32, tag=f"mx{img}", name=f"mx{img}")
    nc.vector.tensor_tensor(
        mx, ob[:, :, 0:2], sb[:, 0:2].unsqueeze(1).to_broadcast([P, NP, 2]),
        op=mybir.AluOpType.max,
    )
    mn = sm.tile([P, NP, 2], F32, tag=f"mn{img}", name=f"mn{img}")
    nc.vector.tensor_tensor(
        mn, ob[:, :, 2:4], sb[:, 2:4].unsqueeze(1).to_broadcast([P, NP, 2]),
        op=mybir.AluOpType.min,
    )
    nc.vector.tensor_sub(mn, mn, mx)
    nc.vector.tensor_scalar_max(mn, mn, 0.0)
    inter = sm.tile([P, NP], F32, tag=f"inter{img}", name=f"inter{img}")
    nc.vector.tensor_mul(inter, mn[:, :, 0], mn[:, :, 1])
    aa2 = sm.tile([P, 2], F32, tag=f"aa2{img}", name=f"aa2{img}")
    nc.vector.tensor_sub(aa2, sb[:, 2:4], sb[:, 0:2])
    aa = sm.tile([P, 1], F32, tag=f"aa{img}", name=f"aa{img}")
    nc.vector.tensor_mul(aa, aa2[:, 0:1], aa2[:, 1:2])
    # suppress iff 0.3 * union < inter  (union = area_sel + areas - inter)
    u = sm.tile([P, NP], F32, tag=f"u{img}", name=f"u{img}")
    nc.vector.scalar_tensor_tensor(
        out=u, in0=ar, scalar=aa[:, 0:1], in1=inter,
        op0=mybir.AluOpType.add, op1=mybir.AluOpType.subtract,
    )
    sup = sm.tile([P, NP], U8, tag=f"sup{img}", name=f"sup{img}")
    nc.vector.scalar_tensor_tensor(
        out=sup, in0=u, scalar=NMS_T, in1=inter,
        op0=mybir.AluOpType.mult, op1=mybir.AluOpType.is_lt,
    )
    nc.vector.copy_predicated(sc, sup, negs)
    # kill the selected entry itself (covers zero-area self-IoU)
    msk2u = sm.tile([P, NP], U8, tag=f"msk2u{img}", name=f"msk2u{img}")
    nc.vector.tensor_tensor(msk2u, gidxn, gmx.to_broadcast([P, NP]),
                            op=mybir.AluOpType.is_equal)
    nc.vector.copy_predicated(sc, msk2u, negs)


def build_nc():
    nc = bacc.Bacc("TRN2", target_bir_lowering=False)
    rois_t = nc.dram_tensor("rois", [BPC, N, 4], F32, kind="ExternalInput")
    probs_t = nc.dram_tensor("probs", [BPC, N, C], F32, kind="ExternalInput")
    deltas_t = nc.dram_tensor("deltas", [BPC, N, C, 4], F32, kind="ExternalInput")
    out_t = nc.dram_tensor("out", [BPC, K, 6], F32, kind="ExternalOutput")

    def out_ap(img):
        return out_t[img].rearrange("k s -> (k s)").unsqueeze(0)

    with TileContext(nc) as tc:
        with (
            tc.tile_pool(name="const", bufs=1) as cpool,
            tc.tile_pool(name="big", bufs=1) as big,
            tc.tile_pool(name="small", bufs=1) as sm,
            tc.tile_pool(name="psum", bufs=1, space="PSUM") as pp,
        ):
            pools = (cpool, big, sm)
            state = {}

            # ---- fast path: probs load (chunked, both DGE queues) ----
            for img in range(BPC):
                pt = big.tile([P, NP, C], F32, tag=f"probs{img}",
                              name=f"probs{img}")
                probs_ap = probs_t[img].rearrange("(p n) c -> p n c", p=P)
                eng = nc.sync if img == 0 else nc.scalar
                o = 0
                for ch in CHUNKS:
                    sl = slice(o, o + ch)
                    eng.dma_start(out=pt[:, sl], in_=probs_ap[:, sl])
                    o += ch
                state[f"probs{img}"] = pt

            # per-chunk class max, interleaved across images so the DVE
            # consumes chunks in arrival order and overlaps the DMA; the
            # confidence-gate count accumulates per chunk as well
            scores = sm.tile([P, BPC, NP], F32, tag="scores", name="scores")
            ge = sm.tile([P, BPC, NP], F32, tag="ge", name="ge")
            cnt = sm.tile([P, len(CHUNKS)], F32, tag="cnt", name="cnt")
            o = 0
            for k, ch in enumerate(CHUNKS):
                sl = slice(o, o + ch)
                for img in range(BPC):
                    pt = state[f"probs{img}"]
                    nc.vector.reduce_max(scores[:, img, sl], pt[:, sl],
                                         axis=mybir.AxisListType.X)
                nc.vector.tensor_scalar(
                    out=ge[:, :, sl], in0=scores[:, :, sl],
                    scalar1=MIN_CONF, scalar2=None,
                    op0=mybir.AluOpType.is_ge, op1=mybir.AluOpType.add,
                    accum_out=cnt[:, k : k + 1],
                )
                o += ch
            state["scores"] = scores
            state["ge"] = ge
            cnt1 = sm.tile([P, 1], F32, tag="cnt1", name="cnt1")
            nc.vector.reduce_sum(cnt1, cnt, axis=mybir.AxisListType.X)

            ones = cpool.tile([P, 1], F32, tag="ones", name="ones")
            nc.vector.memset(ones, 1.0)
            cntp = pp.tile([1, 1], F32, tag="cntp", name="cntp")
            nc.tensor.matmul(cntp, ones, cnt1, start=True, stop=True)
            cnti = sm.tile([1, 1], I32, tag="cnti", name="cnti")
            nc.vector.tensor_copy(cnti, cntp)

            # merged det accumulator for both images; its zeroed state is
            # DMA'd out early and unconditionally (the If branch overwrites
            # the DRAM range later when detections exist)
            out_all = out_t.rearrange("b k s -> (b k s)").unsqueeze(0)
            dets = sm.tile([1, BPC * K * 6], F32, tag="dets", name="dets")
            nc.vector.memset(dets, 0.0)
            nc.sync.dma_start(out=out_all, in_=dets)

            # NMS-loop state tiles (allocated once, written in the If branch)
            for img in range(BPC):
                state[f"sc{img}"] = sm.tile([P, NP], F32, tag=f"sc{img}",
                                            name=f"sc{img}")
                state[f"ob{img}"] = sm.tile([P, NP, 4], F32, tag=f"ob{img}",
                                            name=f"ob{img}")
                state[f"ar{img}"] = sm.tile([P, NP], F32, tag=f"ar{img}",
                                            name=f"ar{img}")
                state[f"cat{img}"] = sm.tile([P, NP, 6], F32, tag=f"cat{img}",
                                             name=f"cat{img}")
            crev = cpool.tile([P, NP, C], F32, tag="crev", name="crev")
            negs = cpool.tile([P, NP], F32, tag="negs", name="negs")
            gidxn = cpool.tile([P, NP], F32, tag="gidxn", name="gidxn")
            state["crev"] = crev
            state["negs"] = negs
            state["gidxn"] = gidxn

            rv = nc.values_load(cnti, min_val=0, max_val=2 * N,
                                skip_runtime_bounds_check=True)

            # ---- slow path: only when at least one roi passes the gate ----
            with tc.If(rv > 0, preferred_fallthrough_block=False,
                       name="slow"):
                # constants used only here
                nc.gpsimd.iota(crev, pattern=[[0, NP], [-1, C]], base=C - 1,
                               channel_multiplier=0,
                               allow_small_or_imprecise_dtypes=True)
                nc.gpsimd.memset(negs, NEG)
                # gidxn[p, n] = -(p*NP + n)  (negated global roi index)
                nc.gpsimd.iota(gidxn, pattern=[[-1, NP]], base=0,
                               channel_multiplier=-NP,
                               allow_small_or_imprecise_dtypes=True)
                for img in range(BPC):
                    _refine_image(nc, tc, pools, img, rois_t, deltas_t, state)

                # merged NMS loop: both images per iteration
                with tc.For_i(0, smin(rv, K), name="nms") as i:
                    for img in range(BPC):
                        _nms_iter(nc, tc, pools, img, state, i, dets,
                                  img * K * 6)

                # real output, overwrites the early zero fill
                nc.sync.dma_start(out=out_all, in_=dets)
    nc.compile()
    return nc


LAST_RESULTS = None  # BassKernelResults of the most recent kernel() call


def kernel(rois, probs, deltas):
    global LAST_RESULTS
    from concourse import bass_utils

    nc = build_nc()
    in_maps = []
    for c in range(NCORES):
        sl = slice(c * BPC, (c + 1) * BPC)
        in_maps.append({
            "rois": np.ascontiguousarray(rois[sl], dtype=np.float32),
            "probs": np.ascontiguousarray(probs[sl], dtype=np.float32),
            "deltas": np.ascontiguousarray(deltas[sl], dtype=np.float32),
        })
    res = bass_utils.run_bass_kernel_spmd(nc, in_maps, core_ids=list(range(NCORES)))
    LAST_RESULTS = res
    return np.concatenate([r["out"] for r in res.results], axis=0)


if __name__ == "__main__":
    rng = np.random.default_rng(0)
    out = kernel(
        rng.random((B, N, 4), np.float32),
        rng.random((B, N, C), np.float32),
        rng.standard_normal((B, N, C, 4)).astype(np.float32),
    )
    print(out.shape, np.abs(out).max())


# revision 19
# speedup vs baseline: 1.0071x; 1.0071x over previous
"""Trainium2 Bass kernel for DetectionLayer (refine + per-class NMS).

Contract: kernel(rois, probs, deltas) with FULL inputs
  rois   [16, 4096, 4]   f32
  probs  [16, 4096, 81]  f32
  deltas [16, 4096, 81, 4] f32
returns [16, 100, 6] f32 detections, matching the jax reference.

Sharding: pure data parallel - 2 images per core across 8 NeuronCores.

Structure (optimized for the data-dependent early exit):
  fast path (always): chunked probs load on both HW-DGE queues, per-chunk
    class-max pipelined with the DMA (img0 on Vector, img1 on GpSimd),
    one fused confidence-gate count, one values_load, early zero-fill of
    the output.
  tc.If(count > 0): deltas+rois load, exact per-argmax delta select, box
    refine, then the merged NMS For_i (both images per iteration) and the
    real output DMAs. When no roi passes min-confidence the whole branch
    is skipped by a register compare - no loop-exit ladders, no DMA.
"""

import numpy as np

import concourse.bacc as bacc
import concourse.bass as bass
import concourse.bass_isa as bass_isa
import concourse.mybir as mybir
from concourse.expressions import smin
from concourse.tile import TileContext

B = 16              # full batch
NCORES = 8
BPC = B // NCORES   # images per core
N = 4096            # rois per image
C = 81              # classes
K = 100             # detection_max_instances
P = 128             # SBUF partitions
NP = N // P         # rois per partition (32)
CHUNKS = (12, 12, 8)  # probs DMA chunk sizes (rois per partition); tail
                      # chunks shrink so the last class-max is off the DMA
                      # critical path while descriptors stay DMA-efficient
NEG = -1e9
MIN_CONF = 0.7
NMS_T = 0.3
F32 = mybir.dt.float32
I32 = mybir.dt.int32
U8 = mybir.dt.uint8


def _refine_image(nc, tc, pools, img, rois_t, deltas_t, state):
    """Deltas select, box refine, NMS-state build for one image.
    Emitted inside the If(count > 0) branch."""
    cpool, big, sm = pools
    pt = state[f"probs{img}"]
    scores = state["scores"][:, img]   # [P, NP]
    ge = state["ge"][:, img]           # [P, NP]
    crev = state["crev"]
    negs = state["negs"]
    sc = state[f"sc{img}"]
    ob = state[f"ob{img}"]
    ar = state[f"ar{img}"]
    cat = state[f"cat{img}"]

    rois_ap = rois_t[img].rearrange("(p n) k -> p n k", p=P)          # [128,32,4]
    deltas_ap = deltas_t[img].rearrange("(p n) c k -> p n c k", p=P)

    dt_ = big.tile([P, NP, C, 4], F32, tag=f"deltas{img}", name=f"deltas{img}")
    # split the 5.3MB load across both HW-DGE queues
    for s in range(8):
        sl = slice(16 * s, 16 * s + 16)
        eng = nc.sync if (s % 2 == 0) else nc.scalar
        eng.dma_start(out=dt_[sl], in_=deltas_ap[sl])
    rt = sm.tile([P, NP, 4], F32, tag=f"rois{img}", name=f"rois{img}")
    nc.sync.dma_start(out=rt, in_=rois_ap)

    # class id = 80 - max((80-c) * (probs == score))  (ties -> smallest c,
    # like argmax). m is built in place over the probs tile.
    m = pt
    nc.vector.tensor_tensor(
        m, pt, scores.unsqueeze(2).to_broadcast([P, NP, C]),
        op=mybir.AluOpType.is_equal,
    )
    nc.vector.tensor_tensor(m, m, crev, op=mybir.AluOpType.mult)
    cid = sm.tile([P, NP], F32, tag=f"cid{img}", name=f"cid{img}")
    nc.vector.reduce_max(cid, m, axis=mybir.AxisListType.X)
    nc.vector.tensor_scalar(
        out=cid, in0=cid, scalar1=-1.0, scalar2=float(C - 1),
        op0=mybir.AluOpType.mult, op1=mybir.AluOpType.add,
    )

    # exact one-hot at the argmax class (safe under intra-roi ties):
    # m2 = (crev == 80 - cid), rebuilt over the probs tile
    t80 = sm.tile([P, NP], F32, tag=f"t80{img}", name=f"t80{img}")
    nc.vector.tensor_scalar(
        out=t80, in0=cid, scalar1=-1.0, scalar2=float(C - 1),
        op0=mybir.AluOpType.mult, op1=mybir.AluOpType.add,
    )
    nc.vector.tensor_tensor(
        m, crev, t80.unsqueeze(2).to_broadcast([P, NP, C]),
        op=mybir.AluOpType.is_equal,
    )

    # select argmax-class delta: deltas *= m2 (bcast over k), sum over c
    d_perm = dt_.rearrange("p n c k -> p n k c")
    nc.vector.tensor_tensor(
        d_perm, d_perm, m.unsqueeze(2).to_broadcast([P, NP, 4, C]),
        op=mybir.AluOpType.mult,
    )
    dsel = sm.tile([P, NP, 4], F32, tag=f"dsel{img}", name=f"dsel{img}")
    nc.vector.reduce_sum(dsel, d_perm, axis=mybir.AxisListType.X)

    # bbox_std scaling (match reference op order exactly)
    nc.vector.tensor_scalar_mul(dsel[:, :, 0:2], dsel[:, :, 0:2], 0.1)
    nc.vector.tensor_scalar_mul(dsel[:, :, 2:4], dsel[:, :, 2:4], 0.2)

    # ---- apply deltas + clip (mirrors _apply_deltas fp32 op order) ----
    h = sm.tile([P, NP], F32, tag=f"h{img}", name=f"h{img}")
    w = sm.tile([P, NP], F32, tag=f"w{img}", name=f"w{img}")
    nc.vector.tensor_sub(h, rt[:, :, 2], rt[:, :, 0])
    nc.vector.tensor_sub(w, rt[:, :, 3], rt[:, :, 1])
    t1 = sm.tile([P, NP], F32, tag=f"t1{img}", name=f"t1{img}")
    t2 = sm.tile([P, NP], F32, tag=f"t2{img}", name=f"t2{img}")
    cy = sm.tile([P, NP], F32, tag=f"cy{img}", name=f"cy{img}")
    cx = sm.tile([P, NP], F32, tag=f"cx{img}", name=f"cx{img}")
    # cy = y1 + 0.5*h + dy*h
    nc.vector.tensor_scalar_mul(t1, h, 0.5)
    nc.vector.tensor_add(t2, rt[:, :, 0], t1)
    nc.vector.tensor_mul(t1, dsel[:, :, 0], h)
    nc.vector.tensor_add(cy, t2, t1)
    # cx = x1 + 0.5*w + dx*w
    nc.vector.tensor_scalar_mul(t1, w, 0.5)
    nc.vector.tensor_add(t2, rt[:, :, 1], t1)
    nc.vector.tensor_mul(t1, dsel[:, :, 1], w)
    nc.vector.tensor_add(cx, t2, t1)
    # h *= exp(dh); w *= exp(dw)
    e = sm.tile([P, NP], F32, tag=f"e{img}", name=f"e{img}")
    nc.scalar.activation(e, dsel[:, :, 2], mybir.ActivationFunctionType.Exp)
    nc.vector.tensor_mul(h, h, e)
    nc.scalar.activation(e, dsel[:, :, 3], mybir.ActivationFunctionType.Exp)
    nc.vector.tensor_mul(w, w, e)

    ref = sm.tile([P, NP, 4], F32, tag=f"ref{img}", name=f"ref{img}")
    nc.vector.tensor_scalar_mul(t1, h, 0.5)
    nc.vector.tensor_sub(ref[:, :, 0], cy, t1)
    nc.vector.tensor_add(ref[:, :, 2], cy, t1)
    nc.vector.tensor_scalar_mul(t2, w, 0.5)
    nc.vector.tensor_sub(ref[:, :, 1], cx, t2)
    nc.vector.tensor_add(ref[:, :, 3], cx, t2)
    nc.vector.tensor_scalar(
        out=ref, in0=ref, scalar1=0.0, scalar2=1.0,
        op0=mybir.AluOpType.max, op1=mybir.AluOpType.min,
    )

    # ---- NMS state ----
    # valid = (cid > 0) & (score >= MIN_CONF); sc0 = valid ? score : NEG
    vf = sm.tile([P, NP], F32, tag=f"vf{img}", name=f"vf{img}")
    nc.vector.tensor_single_scalar(vf, cid, 0.5, op=mybir.AluOpType.is_ge)
    v = sm.tile([P, NP], U8, tag=f"v{img}", name=f"v{img}")
    nc.vector.tensor_mul(v, vf, ge)
    nc.vector.tensor_copy(sc, negs)
    nc.vector.copy_predicated(sc, v, scores)

    # offset boxes = ref + 2*cid, per-class NMS trick
    nc.vector.scalar_tensor_tensor(
        out=ob, in0=cid.unsqueeze(2).to_broadcast([P, NP, 4]), scalar=2.0,
        in1=ref, op0=mybir.AluOpType.mult, op1=mybir.AluOpType.add,
    )
    # areas of offset boxes
    ar2 = sm.tile([P, NP, 2], F32, tag=f"ar2{img}", name=f"ar2{img}")
    nc.vector.tensor_sub(ar2, ob[:, :, 2:4], ob[:, :, 0:2])
    nc.vector.tensor_mul(ar, ar2[:, :, 0], ar2[:, :, 1])
    # cat = [ref(4), cid, score] for one-shot row extraction
    nc.vector.tensor_copy(cat[:, :, 0:4], ref)
    nc.vector.tensor_copy(cat[:, :, 4], cid)
    nc.vector.tensor_copy(cat[:, :, 5], scores)


def _nms_iter(nc, tc, pools, img, state, i, det, det_off):
    """One greedy-NMS step for one image (body of the merged NMS loop).
    Tie-break matches argmax: among equal-score candidates the lowest
    global roi index wins (selected via max over negated indices)."""
    cpool, big, sm = pools
    negs = state["negs"]
    gidxn = state["gidxn"]
    sc = state[f"sc{img}"]
    ob = state[f"ob{img}"]
    ar = state[f"ar{img}"]
    cat = state[f"cat{img}"]

    pm = sm.tile([P, 1], F32, tag=f"pm{img}", name=f"pm{img}")
    nc.vector.reduce_max(pm, sc, axis=mybir.AxisListType.X)
    gm = sm.tile([P, 1], F32, tag=f"gm{img}", name=f"gm{img}")
    nc.gpsimd.partition_all_reduce(gm, pm, channels=P,
                                   reduce_op=bass_isa.ReduceOp.max)
    # candidates at the max score (may be several on exact ties)
    msku = sm.tile([P, NP], U8, tag=f"msku{img}", name=f"msku{img}")
    nc.vector.tensor_tensor(msku, sc, gm.to_broadcast([P, NP]),
                            op=mybir.AluOpType.is_equal)
    # pick lowest roi index among candidates: max over (-index | candidate)
    cand = sm.tile([P, NP], F32, tag=f"cand{img}", name=f"cand{img}")
    nc.vector.tensor_copy(cand, negs)
    nc.vector.copy_predicated(cand, msku, gidxn)
    cmx = sm.tile([P, 1], F32, tag=f"cmx{img}", name=f"cmx{img}")
    nc.vector.reduce_max(cmx, cand, axis=mybir.AxisListType.X)
    gmx = sm.tile([P, 1], F32, tag=f"gmx{img}", name=f"gmx{img}")
    nc.gpsimd.partition_all_reduce(gmx, cmx, channels=P,
                                   reduce_op=bass_isa.ReduceOp.max)
    # exact one-hot of the selected candidate
    msk2 = sm.tile([P, NP], F32, tag=f"msk2{img}", name=f"msk2{img}")
    nc.vector.tensor_tensor(msk2, gidxn, gmx.to_broadcast([P, NP]),
                            op=mybir.AluOpType.is_equal)
    # extract its [ref, cid, score] row via masked sum
    mb6 = sm.tile([P, NP, 6], F32, tag=f"mb6{img}", name=f"mb6{img}")
    nc.vector.tensor_tensor(
        mb6, cat, msk2.unsqueeze(2).to_broadcast([P, NP, 6]),
        op=mybir.AluOpType.mult,
    )
    r6p = sm.tile([P, 6], F32, tag=f"r6p{img}", name=f"r6p{img}")
    nc.vector.reduce_sum(r6p, mb6.rearrange("p n k -> p k n"),
                         axis=mybir.AxisListType.X)
    r6 = sm.tile([P, 6], F32, tag=f"r6{img}", name=f"r6{img}")
    nc.gpsimd.partition_all_reduce(r6, r6p, channels=P,
                                   reduce_op=bass_isa.ReduceOp.add)
    # zero the row when scores are exhausted (gm == NEG)
    okm = sm.tile([P, 1], F32, tag=f"okm{img}", name=f"okm{img}")
    nc.vector.tensor_single_scalar(okm, gm, NEG * 0.5,
                                   op=mybir.AluOpType.is_gt)
    nc.vector.tensor_mul(r6, r6, okm.to_broadcast([P, 6]))
    nc.vector.tensor_copy(det[0:1, bass.ds(det_off + i * 6, 6)], r6[0:1, :])

    # selected offset box, replicated on all partitions
    sb = sm.tile([P, 4], F32, tag=f"sb{img}", name=f"sb{img}")
    nc.vector.scalar_tensor_tensor(
        out=sb, in0=r6[:, 4:5].to_broadcast([P, 4]), scalar=2.0,
        in1=r6[:, 0:4], op0=mybir.AluOpType.mult, op1=mybir.AluOpType.add,
    )
    # IoU(selected, all) on offset boxes
    mx = sm.tile([P, NP, 2], F32, tag=f"mx{img}", name=f"mx{img}")
    nc.vector.tensor_tensor(
        mx, ob[:, :, 0:2], sb[:, 0:2].unsqueeze(1).to_broadcast([P, NP, 2]),
        op=mybir.AluOpType.max,
    )
    mn = sm.tile([P, NP, 2], F32, tag=f"mn{img}", name=f"mn{img}")
    nc.vector.tensor_tensor(
        mn, ob[:, :, 2:4], sb[:, 2:4].unsqueeze(1).to_broadcast([P, NP, 2]),
        op=mybir.AluOpType.min,
    )
    nc.vector.tensor_sub(mn, mn, mx)
    nc.vector.tensor_scalar_max(mn, mn, 0.0)
    inter = sm.tile([P, NP], F32, tag=f"inter{img}", name=f"inter{img}")
    nc.vector.tensor_mul(inter, mn[:, :, 0], mn[:, :, 1])
    aa2 = sm.tile([P, 2], F32, tag=f"aa2{img}", name=f"aa2{img}")
    nc.vector.tensor_sub(aa2, sb[:, 2:4], sb[:, 0:2])
    aa = sm.tile([P, 1], F32, tag=f"aa{img}", name=f"aa{img}")
    nc.vector.tensor_mul(aa, aa2[:, 0:1], aa2[:, 1:2])
    # suppress iff 0.3 * union < inter  (union = area_sel + areas - inter)
    u = sm.tile([P, NP], F32, tag=f"u{img}", name=f"u{img}")
    nc.vector.scalar_tensor_tensor(
        out=u, in0=ar, scalar=aa[:, 0:1], in1=inter,
        op0=mybir.AluOpType.add, op1=mybir.AluOpType.subtract,
    )
    sup = sm.tile([P, NP], U8, tag=f"sup{img}", name=f"sup{img}")
    nc.vector.scalar_tensor_tensor(
        out=sup, in0=u, scalar=NMS_T, in1=inter,
        op0=mybir.AluOpType.mult, op1=mybir.AluOpType.is_lt,
    )
    nc.vector.copy_predicated(sc, sup, negs)
    # kill the selected entry itself (covers zero-area self-IoU)
    msk2u = sm.tile([P, NP], U8, tag=f"msk2u{img}", name=f"msk2u{img}")
    nc.vector.tensor_tensor(msk2u, gidxn, gmx.to_broadcast([P, NP]),
                            op=mybir.AluOpType.is_equal)
    nc.vector.copy_predicated(sc, msk2u, negs)


def build_nc():
    nc = bacc.Bacc("TRN2", target_bir_lowering=False)
    rois_t = nc.dram_tensor("rois", [BPC, N, 4], F32, kind="ExternalInput")
    probs_t = nc.dram_tensor("probs", [BPC, N, C], F32, kind="ExternalInput")
    deltas_t = nc.dram_tensor("deltas", [BPC, N, C, 4], F32, kind="ExternalInput")
    out_t = nc.dram_tensor("out", [BPC, K, 6], F32, kind="ExternalOutput")

    def out_ap(img):
        return out_t[img].rearrange("k s -> (k s)").unsqueeze(0)

    with TileContext(nc) as tc:
        with (
            tc.tile_pool(name="const", bufs=1) as cpool,
            tc.tile_pool(name="big", bufs=1) as big,
            tc.tile_pool(name="small", bufs=1) as sm,
            tc.tile_pool(name="psum", bufs=1, space="PSUM") as pp,
        ):
            pools = (cpool, big, sm)
            state = {}

            # merged det accumulator for both images; its zeroed state is
            # DMA'd out first thing on the gpsimd SWDGE ring (completes
            # during the probs load, so the epilogue never waits on it);
            # the If branch overwrites the DRAM range when detections exist
            out_all = out_t.rearrange("b k s -> (b k s)").unsqueeze(0)
            dets = sm.tile([1, BPC * K * 6], F32, tag="dets", name="dets")
            nc.gpsimd.memset(dets, 0.0)
            nc.gpsimd.dma_start(out=out_all, in_=dets)

            # ---- fast path: probs load (chunked, both DGE queues) ----
            for img in range(BPC):
                pt = big.tile([P, NP, C], F32, tag=f"probs{img}",
                              name=f"probs{img}")
                probs_ap = probs_t[img].rearrange("(p n) c -> p n c", p=P)
                eng = nc.sync if img == 0 else nc.scalar
                o = 0
                for ch in CHUNKS:
                    sl = slice(o, o + ch)
                    eng.dma_start(out=pt[:, sl], in_=probs_ap[:, sl])
                    o += ch
                state[f"probs{img}"] = pt

            # per-chunk class max, interleaved across images so the DVE
            # consumes chunks in arrival order and overlaps the DMA; the
            # confidence-gate count accumulates per chunk as well
            scores = sm.tile([P, BPC, NP], F32, tag="scores", name="scores")
            ge = sm.tile([P, BPC, NP], F32, tag="ge", name="ge")
            cnt = sm.tile([P, len(CHUNKS)], F32, tag="cnt", name="cnt")
            o = 0
            for k, ch in enumerate(CHUNKS):
                sl = slice(o, o + ch)
                for img in range(BPC):
                    pt = state[f"probs{img}"]
                    nc.vector.reduce_max(scores[:, img, sl], pt[:, sl],
                                         axis=mybir.AxisListType.X)
                nc.vector.tensor_scalar(
                    out=ge[:, :, sl], in0=scores[:, :, sl],
                    scalar1=MIN_CONF, scalar2=None,
                    op0=mybir.AluOpType.is_ge, op1=mybir.AluOpType.add,
                    accum_out=cnt[:, k : k + 1],
                )
                o += ch
            state["scores"] = scores
            state["ge"] = ge
            cnt1 = sm.tile([P, 1], F32, tag="cnt1", name="cnt1")
            nc.vector.reduce_sum(cnt1, cnt, axis=mybir.AxisListType.X)

            ones = cpool.tile([P, 1], F32, tag="ones", name="ones")
            nc.vector.memset(ones, 1.0)
            cntp = pp.tile([1, 1], F32, tag="cntp", name="cntp")
            nc.tensor.matmul(cntp, ones, cnt1, start=True, stop=True)
            cnti = sm.tile([1, 1], I32, tag="cnti", name="cnti")
            nc.vector.tensor_copy(cnti, cntp)

            # NMS-loop state tiles (allocated once, written in the If branch)
            for img in range(BPC):
                state[f"sc{img}"] = sm.tile([P, NP], F32, tag=f"sc{img}",
                                            name=f"sc{img}")
                state[f"ob{img}"] = sm.tile([P, NP, 4], F32, tag=f"ob{img}",
                                            name=f"ob{img}")
                state[f"ar{img}"] = sm.tile([P, NP], F32, tag=f"ar{img}",
                                            name=f"ar{img}")
                state[f"cat{img}"] = sm.tile([P, NP, 6], F32, tag=f"cat{img}",
                                             name=f"cat{img}")
            crev = cpool.tile([P, NP, C], F32, tag="crev", name="crev")
            negs = cpool.tile([P, NP], F32, tag="negs", name="negs")
            gidxn = cpool.tile([P, NP], F32, tag="gidxn", name="gidxn")
            state["crev"] = crev
            state["negs"] = negs
            state["gidxn"] = gidxn

            rv = nc.values_load(cnti, min_val=0, max_val=2 * N,
                                skip_runtime_bounds_check=True)

            # ---- slow path: only when at least one roi passes the gate ----
            with tc.If(rv > 0, preferred_fallthrough_block=False,
                       name="slow"):
                # constants used only here
                nc.gpsimd.iota(crev, pattern=[[0, NP], [-1, C]], base=C - 1,
                               channel_multiplier=0,
                               allow_small_or_imprecise_dtypes=True)
                nc.gpsimd.memset(negs, NEG)
                # gidxn[p, n] = -(p*NP + n)  (negated global roi index)
                nc.gpsimd.iota(gidxn, pattern=[[-1, NP]], base=0,
                               channel_multiplier=-NP,
                               allow_small_or_imprecise_dtypes=True)
                for img in range(BPC):
                    _refine_image(nc, tc, pools, img, rois_t, deltas_t, state)

                # merged NMS loop: both images per iteration
                with tc.For_i(0, smin(rv, K), name="nms") as i:
                    for img in range(BPC):
                        _nms_iter(nc, tc, pools, img, state, i, dets,
                                  img * K * 6)

                # real output, overwrites the early zero fill
                nc.sync.dma_start(out=out_all, in_=dets)
    nc.compile()
    return nc


LAST_RESULTS = None  # BassKernelResults of the most recent kernel() call


def kernel(rois, probs, deltas):
    global LAST_RESULTS
    from concourse import bass_utils

    nc = build_nc()
    in_maps = []
    for c in range(NCORES):
        sl = slice(c * BPC, (c + 1) * BPC)
        in_maps.append({
            "rois": np.ascontiguousarray(rois[sl], dtype=np.float32),
            "probs": np.ascontiguousarray(probs[sl], dtype=np.float32),
            "deltas": np.ascontiguousarray(deltas[sl], dtype=np.float32),
        })
    res = bass_utils.run_bass_kernel_spmd(nc, in_maps, core_ids=list(range(NCORES)))
    LAST_RESULTS = res
    return np.concatenate([r["out"] for r in res.results], axis=0)


if __name__ == "__main__":
    rng = np.random.default_rng(0)
    out = kernel(
        rng.random((B, N, 4), np.float32),
        rng.random((B, N, C), np.float32),
        rng.standard_normal((B, N, C, 4)).astype(np.float32),
    )
    print(out.shape, np.abs(out).max())
